# revision 59
# baseline (speedup 1.0000x reference)
"""Trainium2 Bass kernel for ContextQueryAttention (BiDAF-style trilinear attention).

Math (per batch):
  S = C@w1 + (Q@w2)^T + (C*w3)@Q^T          [n, m]   (S_true includes the Qw2 row term)
  S_row = softmax_m(S); S_col = softmax_n(S)
  A = S_row @ Q
  B = S_row @ (S_col^T @ C)                  (reassociated: avoids [n,n] intermediate)
  out = [C, A, C*A, C*B]                     [n, 4d]

Key implementation choices vs the f32 baseline:
  - fp16 I/O: C/Q uploaded as fp16, device output [A|CA|CB] stored fp16 and
    upcast on host.  The C passthrough block never crosses the device: kernel()
    concatenates the host-resident C during unshard.  1.03MB DMA per batch.
  - Host prepares the tiny linear input projections Wm = Q^T*w3 + w1 (the
    stationary of the S matmul) and qw2 = Q@w2 (the exp bias); per-rep loads
    (0.25MB + 4KB), removing the Q transpose from the critical path.
  - One exp only: ET = exp(S^T + qw2 bias) [j, n].  The natural orientation
    EN' = ET^T comes from 8 PE transposes; the exp(qw2[j]) column factor it
    carries cancels in the column softmax (T2 = (EN' contracted @ C) / cs where
    cs, the accum_out of the ET exp, carries the same factor).
  - Rowsums via a third matmul per chunk against a ones column packed next to
    the T2 accumulator -> one reciprocal, then per-half tensor_tensor with a
    stride-0-broadcast rr AP normalizes and drains A; B drains unnormalized
    (ACT) with the normalization folded into Crr = C*rr (Pool), CB = B_u*Crr.
  - Software-pipelined emission: batch b+1's front end (C^T transposes, CT
    drain, S matmul, exp) is emitted inside batch b's middle, so the in-order
    PE/ACT queues fill b's dependency stalls with b+1's work.
  - PSUM rings sized so no cross-batch pool edge binds: {st}, {pB}, {pQ half
    tiles}, {ct,en}, {ec|rs} - exactly 8 banks.
  - n indexed as n = 8*p + c: per-partition-contiguous DMA for both the C load
    and the single 6KB-per-partition output store.
  - nreps > 1 (timing multiplier) runs an on-device For_i loop: one dispatch
    carries nreps*8 batches, so device time dominates any host dispatch cost.
  - Sharding: data-parallel over batch, 8 batches per core, no communication.

Measured (this configuration): ~49.5-50.7us per rep steady-state (median-fit,
loop bubble ~13-16us/iteration excluded) vs 51.156us f32 baseline; CoreSim
predicts 45.5us.  Tested-and-rejected on HW: sequential emission (66us),
Q-load consolidation into a per-rep tile (66us), exp/EN half-splits (+0.4us),
CT drain on a single engine (confounded by env drift; sim +1.8us).  Next
levers: free one PSUM bank (split st into two 1-bank halves + two-part exp)
to allow pse bufs=2 and tail-skew emission (delay tail(b-1) past frontB(b));
always A/B-test with paired interleaved measurements in one process - this
environment drifts +-10% on a minutes scale.
"""
import numpy as np

B, N, M, D = 64, 1024, 128, 128
NCORES = 8
BPC = B // NCORES      # batches per core
NCH = N // 128         # 128-row chunks per batch

_CACHE = {}


def _build_program(nreps=1, loop_unroll=None):
    import concourse.tile as tile
    from concourse import bacc, masks, mybir

    fp32 = mybir.dt.float32
    fp16 = mybir.dt.float16
    AL = mybir.AluOpType
    AF = mybir.ActivationFunctionType

    nc = bacc.Bacc("TRN2", target_bir_lowering=False, debug=False, num_devices=NCORES)
    C_d = nc.dram_tensor("Cin", [BPC, N, D], fp16, kind="ExternalInput")
    Q_d = nc.dram_tensor("Qin", [BPC, M, D], fp16, kind="ExternalInput")
    Wm_d = nc.dram_tensor("Wmx", [D, BPC, M], fp16, kind="ExternalInput")
    Qw2_d = nc.dram_tensor("Qw2", [M, BPC], fp32, kind="ExternalInput")
    O_d = nc.dram_tensor("Out", [BPC, N, 3 * D], fp16, kind="ExternalOutput")

    with tile.TileContext(nc) as tc:
        with (
            tc.tile_pool(name="const", bufs=1) as constp,
            tc.tile_pool(name="small", bufs=4) as smallp,
            tc.tile_pool(name="cbuf", bufs=6) as cbufp,
            tc.tile_pool(name="ebuf", bufs=4) as ebufp,
            tc.tile_pool(name="obuf", bufs=4) as obufp,
            tc.tile_pool(name="psS", bufs=1, space="PSUM") as psS,
            tc.tile_pool(name="psB2", bufs=1, space="PSUM") as psB2,
            tc.tile_pool(name="psq", bufs=1, space="PSUM") as psq,
            tc.tile_pool(name="ps16", bufs=2, space="PSUM") as ps16,
            tc.tile_pool(name="pse", bufs=1, space="PSUM") as pse,
        ):
            ident = constp.tile([128, 128], fp16)
            masks.make_identity(nc, ident[:])
            ones1 = constp.tile([128, 1], fp16)
            nc.gpsimd.memset(ones1[:], 1.0)

            def load_inputs(bi):
                """Issue batch bi's input DMAs (prefetched ahead of compute)."""
                b = bi % BPC
                C_sb = cbufp.tile([128, NCH, 128], fp16, tag="csb")
                nc.sync.dma_start(
                    C_sb[:], C_d.ap()[b].rearrange("(p c) d -> p c d", c=NCH)
                )
                qstage = cbufp.tile([128, 256], fp16, tag="qstage")
                nc.sync.dma_start(qstage[:, 0:128], Q_d.ap()[b])
                wq = None
                if b == 0:
                    wm_all = cbufp.tile([128, BPC, 128], fp16, tag="wma")
                    nc.sync.dma_start(wm_all[:], Wm_d.ap())
                    qw2_all = cbufp.tile([128, BPC], fp32, tag="qw2a")
                    nc.sync.dma_start(qw2_all[:], Qw2_d.ap())
                    wq = (wm_all, qw2_all)
                return C_sb, qstage, wq

            def frontA(C_sb):
                """C^T transposes + CT drain (fp16)."""
                ct_ps = ps16.tile([128, NCH, 128], fp16, tag="ps16")
                for c in range(NCH):
                    nc.tensor.transpose(ct_ps[:, c, :], C_sb[:, c, :], ident[:])
                CT = cbufp.tile([128, NCH, 128], fp16, tag="ct")
                nc.scalar.copy(CT[:, 0:4, :], ct_ps[:, 0:4, :])
                nc.vector.tensor_copy(CT[:, 4:8, :], ct_ps[:, 4:8, :])
                return CT

            def frontB(b, CT, wm_all, qw2_all):
                """S^T matmul -> exp -> ET, rcs."""
                CT_flat = CT[:].rearrange("d c p -> d (c p)")
                st_ps = psS.tile([128, NCH, 128], fp32, tag="psS")
                st_flat = st_ps[:].rearrange("m c p -> m (c p)")
                Wm = wm_all[:, b, :]
                nc.tensor.matmul(st_flat[:, 0:512], Wm, CT_flat[:, 0:512])
                nc.tensor.matmul(st_flat[:, 512:1024], Wm, CT_flat[:, 512:1024])
                ET = ebufp.tile([128, NCH, 128], fp16, tag="et")
                cs = smallp.tile([128, 1], fp32, tag="cs")
                nc.scalar.activation(
                    ET[:].rearrange("m c p -> m (c p)"),
                    st_flat,
                    AF.Exp,
                    bias=qw2_all[:, b : b + 1],
                    accum_out=cs[:],
                )
                rcs = smallp.tile([128, 1], fp32, tag="rcs")
                nc.vector.reciprocal(rcs[:], cs[:])
                return ET, rcs

            def mid1(ET):
                """EN' = ET^T per chunk + drain (keeps the qw2 column factor,
                which cancels in the column softmax)."""
                en_ps = ps16.tile([128, NCH, 128], fp16, tag="ps16")
                for c in range(NCH):
                    nc.tensor.transpose(en_ps[:, c, :], ET[:, c, :], ident[:])
                ENs = ebufp.tile([128, NCH, 128], fp16, tag="ens")
                nc.vector.tensor_copy(ENs[:], en_ps[:])
                return ENs

            def mid2(C_sb, qstage, ENs, rcs):
                """T2 = (EN' contracted over n @ C) / cs -> qstage[:, 128:256]."""
                ecrs = pse.tile([128, 136], fp32, tag="pse")
                ec_ps = ecrs[:, 0:128]
                for c in range(NCH):
                    nc.tensor.matmul(
                        ec_ps, ENs[:, c, :], C_sb[:, c, :],
                        start=(c == 0), stop=(c == NCH - 1),
                    )
                nc.scalar.activation(
                    qstage[:, 128:256], ec_ps, AF.Copy, scale=rcs[:]
                )
                return ecrs

            def tail(b, C_sb, qstage, ET, ecrs):
                """EQ/ET2/rowsum matmuls, normalize+drain A, B path, store.

                mmB is emitted first per chunk: PE is in-order and mmB(c=0)
                data-depends on T2s, so the rs writes into ecrs's bank land
                strictly after ACT's ec read (no PE-W/ACT-R bank hazard)."""
                pB = psB2.tile([128, NCH, 128], fp32, tag="psB2")
                rs_ps = ecrs[:, 128:136]
                rrs = smallp.tile([128, NCH], fp32, tag="rrs")
                o_big = obufp.tile([128, NCH, 384], fp16, tag="obig")
                pQ1 = psq.tile([128, 4, 128], fp32, tag="psq")
                for c in range(NCH):
                    nc.tensor.matmul(pB[:, c, :], ET[:, c, :], qstage[:, 128:256])
                    nc.tensor.matmul(rs_ps[:, c : c + 1], ET[:, c, :], ones1[:])
                    if c < 4:
                        nc.tensor.matmul(
                            pQ1[:, c, :], ET[:, c, :], qstage[:, 0:128]
                        )
                nc.vector.reciprocal(rrs[:], rs_ps)
                crr = obufp.tile([128, NCH, 128], fp16, tag="crr")
                rr_bc = rrs[:].unsqueeze(-1).to_broadcast([128, NCH, 128])
                nc.gpsimd.tensor_tensor(crr[:], C_sb[:], rr_bc, AL.mult)
                nc.vector.tensor_tensor(
                    o_big[:, 0:4, 0:128], pQ1[:],
                    rrs[:, 0:4].unsqueeze(-1).to_broadcast([128, 4, 128]),
                    AL.mult,
                )
                pQ2 = psq.tile([128, 4, 128], fp32, tag="psq")
                for c in range(4, NCH):
                    nc.tensor.matmul(pQ2[:, c - 4, :], ET[:, c, :], qstage[:, 0:128])
                nc.vector.tensor_tensor(
                    o_big[:, 4:8, 0:128], pQ2[:],
                    rrs[:, 4:8].unsqueeze(-1).to_broadcast([128, 4, 128]),
                    AL.mult,
                )

                # B plain drain (ACT); Crr/CA/CB on Pool
                bstage = obufp.tile([128, NCH, 128], fp16, tag="bstage")
                nc.scalar.copy(bstage[:], pB[:])
                nc.gpsimd.tensor_mul(o_big[:, :, 128:256], o_big[:, :, 0:128], C_sb[:])
                nc.gpsimd.tensor_mul(o_big[:, :, 256:384], bstage[:], crr[:])

                nc.sync.dma_start(
                    O_d.ap()[b].rearrange("(p c) e -> p c e", c=NCH), o_big[:]
                )

            def body(TOT):
                loads = {0: load_inputs(0)}
                if TOT > 1:
                    loads[1] = load_inputs(1)
                wq_cur = loads[0][2]
                C_sb, qstage = loads[0][:2]
                CT = frontA(C_sb)
                fr = frontB(0, CT, *wq_cur)
                for bi in range(TOT):
                    b = bi % BPC
                    ET, rcs = fr
                    if bi + 2 < TOT:
                        loads[bi + 2] = load_inputs(bi + 2)
                    ENs = mid1(ET)
                    if bi + 1 < TOT:
                        C2, q2, wq = loads.pop(bi + 1)
                        if wq is not None:
                            wq_cur = wq
                        CT2 = frontA(C2)
                    ecrs = mid2(C_sb, qstage, ENs, rcs)
                    if bi + 1 < TOT:
                        fr = frontB((bi + 1) % BPC, CT2, *wq_cur)
                    tail(b, C_sb, qstage, ET, ecrs)
                    if bi + 1 < TOT:
                        C_sb, qstage = C2, q2

            # nreps=1 (the production path) stays a flat unrolled program; for
            # timing multipliers, run an on-device loop so one dispatch carries
            # nreps * BPC batches without growing the instruction stream.
            if nreps == 1:
                body(BPC)
            else:
                R = 4 if nreps % 4 == 0 else (2 if nreps % 2 == 0 else 1)
                if loop_unroll is not None and nreps % loop_unroll == 0:
                    R = loop_unroll
                with tc.For_i(0, nreps // R):
                    body(R * BPC)

    nc.compile()
    return nc


def _in_maps(C, Q, W):
    C16 = C.astype(np.float16)
    Q16 = Q.astype(np.float16)
    w1, w3 = W[:D], W[2 * D :]
    w2 = W[D : 2 * D]
    # Wm[b, d, j] = Q[b, j, d] * w3[d] + w1[d]; shipped as [D, BPC, M]
    Wm = Q.transpose(0, 2, 1) * w3[None, :, None] + w1[None, :, None]  # [B, D, M]
    qw2 = Q @ w2  # [B, M]
    maps = []
    for i in range(NCORES):
        sl = slice(i * BPC, (i + 1) * BPC)
        maps.append(
            {
                "Cin": C16[sl],
                "Qin": Q16[sl],
                "Wmx": np.ascontiguousarray(
                    Wm[sl].transpose(1, 0, 2)
                ).astype(np.float16),
                "Qw2": np.ascontiguousarray(qw2[sl].T).astype(np.float32),
            }
        )
    return maps


def kernel(C, Q, W):
    from concourse.bass_utils import run_bass_kernel_spmd

    if "nc" not in _CACHE:
        _CACHE["nc"] = _build_program()
    nc = _CACHE["nc"]

    C = np.ascontiguousarray(C, dtype=np.float32)
    Q = np.ascontiguousarray(Q, dtype=np.float32)
    W = np.ascontiguousarray(W, dtype=np.float32)
    res = run_bass_kernel_spmd(nc, _in_maps(C, Q, W), core_ids=list(range(NCORES)))
    _CACHE["last_result"] = res
    acb = np.concatenate([r["Out"] for r in res.results], axis=0).astype(np.float32)
    out = np.empty((B, N, 4 * D), dtype=np.float32)
    out[:, :, 0:D] = C
    out[:, :, D:] = acb
    return out


# revision 60
# speedup vs baseline: 1.0169x; 1.0169x over previous
"""Trainium2 Bass kernel for ContextQueryAttention (BiDAF-style trilinear attention).

Math (per batch):
  S = C@w1 + (Q@w2)^T + (C*w3)@Q^T          [n, m]   (S_true includes the Qw2 row term)
  S_row = softmax_m(S); S_col = softmax_n(S)
  A = S_row @ Q
  B = S_row @ (S_col^T @ C)                  (reassociated: avoids [n,n] intermediate)
  out = [C, A, C*A, C*B]                     [n, 4d]

Key implementation choices vs the f32 baseline:
  - fp16 I/O: C/Q uploaded as fp16, device output [A|CA|CB] stored fp16 and
    upcast on host.  The C passthrough block never crosses the device: kernel()
    concatenates the host-resident C during unshard.  1.03MB DMA per batch.
  - Host prepares the tiny linear input projections Wm = Q^T*w3 + w1 (the
    stationary of the S matmul) and qw2 = Q@w2 (the exp bias); per-rep loads
    (0.25MB + 4KB), removing the Q transpose from the critical path.
  - One exp only: ET = exp(S^T + qw2 bias) [j, n].  The natural orientation
    EN' = ET^T comes from 8 PE transposes; the exp(qw2[j]) column factor it
    carries cancels in the column softmax (T2 = (EN' contracted @ C) / cs where
    cs, the accum_out of the ET exp, carries the same factor).
  - Rowsums via a third matmul per chunk against a ones column packed next to
    the T2 accumulator -> one reciprocal, then per-half tensor_tensor with a
    stride-0-broadcast rr AP normalizes and drains A; B drains unnormalized
    (ACT) with the normalization folded into Crr = C*rr (Pool), CB = B_u*Crr.
  - Software-pipelined emission: batch b+1's front end (C^T transposes, CT
    drain, S matmul, exp) is emitted inside batch b's middle, so the in-order
    PE/ACT queues fill b's dependency stalls with b+1's work.
  - PSUM rings sized so no cross-batch pool edge binds: {st}, {pB}, {pQ half
    tiles}, {ct,en}, {ec|rs} - exactly 8 banks.
  - n indexed as n = 8*p + c: per-partition-contiguous DMA for both the C load
    and the single 6KB-per-partition output store.
  - nreps > 1 (timing multiplier) runs an on-device For_i loop: one dispatch
    carries nreps*8 batches, so device time dominates any host dispatch cost.
  - Sharding: data-parallel over batch, 8 batches per core, no communication.

Measured (this configuration): ~49.5-50.7us per rep steady-state (median-fit,
loop bubble ~13-16us/iteration excluded) vs 51.156us f32 baseline; CoreSim
predicts 45.5us.  Tested-and-rejected on HW: sequential emission (66us),
Q-load consolidation into a per-rep tile (66us), exp/EN half-splits (+0.4us),
CT drain on a single engine (confounded by env drift; sim +1.8us).  Next
levers: free one PSUM bank (split st into two 1-bank halves + two-part exp)
to allow pse bufs=2 and tail-skew emission (delay tail(b-1) past frontB(b));
always A/B-test with paired interleaved measurements in one process - this
environment drifts +-10% on a minutes scale.
"""
import numpy as np

B, N, M, D = 64, 1024, 128, 128
NCORES = 8
BPC = B // NCORES      # batches per core
NCH = N // 128         # 128-row chunks per batch

_CACHE = {}


def _build_program(nreps=1, loop_unroll=None):
    import concourse.tile as tile
    from concourse import bacc, masks, mybir

    fp32 = mybir.dt.float32
    fp16 = mybir.dt.float16
    AL = mybir.AluOpType
    AF = mybir.ActivationFunctionType

    nc = bacc.Bacc("TRN2", target_bir_lowering=False, debug=False, num_devices=NCORES)
    C_d = nc.dram_tensor("Cin", [BPC, N, D], fp16, kind="ExternalInput")
    Q_d = nc.dram_tensor("Qin", [BPC, M, D], fp16, kind="ExternalInput")
    Wm_d = nc.dram_tensor("Wmx", [D, BPC, M], fp16, kind="ExternalInput")
    Qw2_d = nc.dram_tensor("Qw2", [M, BPC], fp32, kind="ExternalInput")
    O_d = nc.dram_tensor("Out", [BPC, N, 3 * D], fp16, kind="ExternalOutput")

    with tile.TileContext(nc) as tc:
        with (
            tc.tile_pool(name="const", bufs=1) as constp,
            tc.tile_pool(name="small", bufs=6) as smallp,
            tc.tile_pool(name="cbuf", bufs=6) as cbufp,
            tc.tile_pool(name="ebuf", bufs=6) as ebufp,
            tc.tile_pool(name="obuf", bufs=6) as obufp,
            tc.tile_pool(name="psS", bufs=1, space="PSUM") as psS,
            tc.tile_pool(name="psB2", bufs=1, space="PSUM") as psB2,
            tc.tile_pool(name="psq", bufs=1, space="PSUM") as psq,
            tc.tile_pool(name="ps16", bufs=2, space="PSUM") as ps16,
            tc.tile_pool(name="pse", bufs=1, space="PSUM") as pse,
        ):
            ident = constp.tile([128, 128], fp16)
            masks.make_identity(nc, ident[:])
            ones1 = constp.tile([128, 1], fp16)
            nc.gpsimd.memset(ones1[:], 1.0)

            def load_inputs(bi):
                """Issue batch bi's input DMAs (prefetched ahead of compute)."""
                b = bi % BPC
                C_sb = cbufp.tile([128, NCH, 128], fp16, tag="csb")
                nc.sync.dma_start(
                    C_sb[:], C_d.ap()[b].rearrange("(p c) d -> p c d", c=NCH)
                )
                qstage = cbufp.tile([128, 256], fp16, tag="qstage")
                nc.sync.dma_start(qstage[:, 0:128], Q_d.ap()[b])
                wq = None
                if b == 0:
                    wm_all = cbufp.tile([128, BPC, 128], fp16, tag="wma")
                    nc.sync.dma_start(wm_all[:], Wm_d.ap())
                    qw2_all = cbufp.tile([128, BPC], fp32, tag="qw2a")
                    nc.sync.dma_start(qw2_all[:], Qw2_d.ap())
                    wq = (wm_all, qw2_all)
                return C_sb, qstage, wq

            def frontA(C_sb):
                """C^T transposes + CT drain (fp16)."""
                ct_ps = ps16.tile([128, NCH, 128], fp16, tag="ps16")
                for c in range(NCH):
                    nc.tensor.transpose(ct_ps[:, c, :], C_sb[:, c, :], ident[:])
                CT = cbufp.tile([128, NCH, 128], fp16, tag="ct")
                nc.scalar.copy(CT[:, 0:4, :], ct_ps[:, 0:4, :])
                nc.vector.tensor_copy(CT[:, 4:8, :], ct_ps[:, 4:8, :])
                return CT

            def frontB(b, CT, wm_all, qw2_all):
                """S^T matmul -> exp -> ET, rcs."""
                CT_flat = CT[:].rearrange("d c p -> d (c p)")
                st_ps = psS.tile([128, NCH, 128], fp32, tag="psS")
                st_flat = st_ps[:].rearrange("m c p -> m (c p)")
                Wm = wm_all[:, b, :]
                nc.tensor.matmul(st_flat[:, 0:512], Wm, CT_flat[:, 0:512])
                nc.tensor.matmul(st_flat[:, 512:1024], Wm, CT_flat[:, 512:1024])
                ET = ebufp.tile([128, NCH, 128], fp16, tag="et")
                cs = smallp.tile([128, 1], fp32, tag="cs")
                nc.scalar.activation(
                    ET[:].rearrange("m c p -> m (c p)"),
                    st_flat,
                    AF.Exp,
                    bias=qw2_all[:, b : b + 1],
                    accum_out=cs[:],
                )
                rcs = smallp.tile([128, 1], fp32, tag="rcs")
                nc.vector.reciprocal(rcs[:], cs[:])
                return ET, rcs

            def mid1(ET):
                """EN' = ET^T per chunk + drain (keeps the qw2 column factor,
                which cancels in the column softmax)."""
                en_ps = ps16.tile([128, NCH, 128], fp16, tag="ps16")
                for c in range(NCH):
                    nc.tensor.transpose(en_ps[:, c, :], ET[:, c, :], ident[:])
                ENs = ebufp.tile([128, NCH, 128], fp16, tag="ens")
                nc.vector.tensor_copy(ENs[:], en_ps[:])
                return ENs

            def mid2(C_sb, qstage, ENs, rcs):
                """T2 = (EN' contracted over n @ C) / cs -> qstage[:, 128:256]."""
                ecrs = pse.tile([128, 136], fp32, tag="pse")
                ec_ps = ecrs[:, 0:128]
                for c in range(NCH):
                    nc.tensor.matmul(
                        ec_ps, ENs[:, c, :], C_sb[:, c, :],
                        start=(c == 0), stop=(c == NCH - 1),
                    )
                nc.scalar.activation(
                    qstage[:, 128:256], ec_ps, AF.Copy, scale=rcs[:]
                )
                return ecrs

            def tail(b, C_sb, qstage, ET, ecrs):
                """EQ/ET2/rowsum matmuls, normalize+drain A, B path, store.

                mmB is emitted first per chunk: PE is in-order and mmB(c=0)
                data-depends on T2s, so the rs writes into ecrs's bank land
                strictly after ACT's ec read (no PE-W/ACT-R bank hazard)."""
                pB = psB2.tile([128, NCH, 128], fp32, tag="psB2")
                rs_ps = ecrs[:, 128:136]
                rrs = smallp.tile([128, NCH], fp32, tag="rrs")
                o_big = obufp.tile([128, NCH, 384], fp16, tag="obig")
                pQ1 = psq.tile([128, 4, 128], fp32, tag="psq")
                for c in range(NCH):
                    nc.tensor.matmul(pB[:, c, :], ET[:, c, :], qstage[:, 128:256])
                    nc.tensor.matmul(rs_ps[:, c : c + 1], ET[:, c, :], ones1[:])
                    if c < 4:
                        nc.tensor.matmul(
                            pQ1[:, c, :], ET[:, c, :], qstage[:, 0:128]
                        )
                nc.vector.reciprocal(rrs[:], rs_ps)
                crr = obufp.tile([128, NCH, 128], fp16, tag="crr")
                rr_bc = rrs[:].unsqueeze(-1).to_broadcast([128, NCH, 128])
                nc.gpsimd.tensor_tensor(crr[:], C_sb[:], rr_bc, AL.mult)
                nc.vector.tensor_tensor(
                    o_big[:, 0:4, 0:128], pQ1[:],
                    rrs[:, 0:4].unsqueeze(-1).to_broadcast([128, 4, 128]),
                    AL.mult,
                )
                pQ2 = psq.tile([128, 4, 128], fp32, tag="psq")
                for c in range(4, NCH):
                    nc.tensor.matmul(pQ2[:, c - 4, :], ET[:, c, :], qstage[:, 0:128])
                nc.vector.tensor_tensor(
                    o_big[:, 4:8, 0:128], pQ2[:],
                    rrs[:, 4:8].unsqueeze(-1).to_broadcast([128, 4, 128]),
                    AL.mult,
                )

                # B plain drain (ACT); Crr/CA/CB on Pool
                bstage = obufp.tile([128, NCH, 128], fp16, tag="bstage")
                nc.scalar.copy(bstage[:], pB[:])
                nc.gpsimd.tensor_mul(o_big[:, :, 128:256], o_big[:, :, 0:128], C_sb[:])
                nc.gpsimd.tensor_mul(o_big[:, :, 256:384], bstage[:], crr[:])

                nc.sync.dma_start(
                    O_d.ap()[b].rearrange("(p c) e -> p c e", c=NCH), o_big[:]
                )

            def body(TOT):
                loads = {0: load_inputs(0)}
                if TOT > 1:
                    loads[1] = load_inputs(1)
                wq_cur = loads[0][2]
                C_sb, qstage = loads[0][:2]
                CT = frontA(C_sb)
                fr = frontB(0, CT, *wq_cur)
                for bi in range(TOT):
                    b = bi % BPC
                    ET, rcs = fr
                    if bi + 2 < TOT:
                        loads[bi + 2] = load_inputs(bi + 2)
                    ENs = mid1(ET)
                    if bi + 1 < TOT:
                        C2, q2, wq = loads.pop(bi + 1)
                        if wq is not None:
                            wq_cur = wq
                        CT2 = frontA(C2)
                    ecrs = mid2(C_sb, qstage, ENs, rcs)
                    if bi + 1 < TOT:
                        fr = frontB((bi + 1) % BPC, CT2, *wq_cur)
                    tail(b, C_sb, qstage, ET, ecrs)
                    if bi + 1 < TOT:
                        C_sb, qstage = C2, q2

            # nreps=1 (the production path) stays a flat unrolled program; for
            # timing multipliers, run an on-device loop so one dispatch carries
            # nreps * BPC batches without growing the instruction stream.
            if nreps == 1:
                body(BPC)
            else:
                R = 4 if nreps % 4 == 0 else (2 if nreps % 2 == 0 else 1)
                if loop_unroll is not None and nreps % loop_unroll == 0:
                    R = loop_unroll
                with tc.For_i(0, nreps // R):
                    body(R * BPC)

    nc.compile()
    return nc


def _in_maps(C, Q, W):
    C16 = C.astype(np.float16)
    Q16 = Q.astype(np.float16)
    w1, w3 = W[:D], W[2 * D :]
    w2 = W[D : 2 * D]
    # Wm[b, d, j] = Q[b, j, d] * w3[d] + w1[d]; shipped as [D, BPC, M]
    Wm = Q.transpose(0, 2, 1) * w3[None, :, None] + w1[None, :, None]  # [B, D, M]
    qw2 = Q @ w2  # [B, M]
    maps = []
    for i in range(NCORES):
        sl = slice(i * BPC, (i + 1) * BPC)
        maps.append(
            {
                "Cin": C16[sl],
                "Qin": Q16[sl],
                "Wmx": np.ascontiguousarray(
                    Wm[sl].transpose(1, 0, 2)
                ).astype(np.float16),
                "Qw2": np.ascontiguousarray(qw2[sl].T).astype(np.float32),
            }
        )
    return maps


def kernel(C, Q, W):
    from concourse.bass_utils import run_bass_kernel_spmd

    if "nc" not in _CACHE:
        _CACHE["nc"] = _build_program()
    nc = _CACHE["nc"]

    C = np.ascontiguousarray(C, dtype=np.float32)
    Q = np.ascontiguousarray(Q, dtype=np.float32)
    W = np.ascontiguousarray(W, dtype=np.float32)
    res = run_bass_kernel_spmd(nc, _in_maps(C, Q, W), core_ids=list(range(NCORES)))
    _CACHE["last_result"] = res
    acb = np.concatenate([r["Out"] for r in res.results], axis=0).astype(np.float32)
    out = np.empty((B, N, 4 * D), dtype=np.float32)
    out[:, :, 0:D] = C
    out[:, :, D:] = acb
    return out
